# revision 25
# baseline (speedup 1.0000x reference)
"""Trainium2 Bass kernel for the windowed bidirectional LSTM encoder.

Semantics (derived from the reference): each direction is a plain LSTM cell
chain over a token stream of length NT = 2S-1 (windows overlap, so tokens
repeat: fwd stream = x0,x1,x1,x2,x2,...,x511; bwd stream = x1,x0,x2,x1,...).
The output is the per-feature max of all NT hidden states of each direction,
concatenated: emb = [max_t h_f(t) | max_t h_b(t)] -> (B, 2H).

Distribution: 8 cores, each owns 8 batch rows and runs both directions.

Chunk-parallel recurrence: the LSTM map is contractive (forget gate < 0.82),
so the NT-step chain is split into K=16 chunks of L=64 steps processed in
parallel as extra "batch" columns. Each chunk is warmed up W=12 steps from a
zero state starting inside the previous chunk's range; warmed-up state error
is ~0.6^W ~ 1e-3. Chunk 0's warmup reads a zero-padded P region, which keeps
its state exactly (0,0) until its true start.

Per-core phases (GPSIMD compute is unavailable on this image, so everything
pointwise lives on DVE + ACT):
  phase 1: P[d,g,tok,b] = bf16(Wih_d @ x_tok + bias_d) for all 512 tokens
           (weights-stationary matmuls over piece-wise DMA'd X; bias folded
           into the PSUM->SBUF copies, which alternate ACT/DVE).
  phase 2: NP = L+W parallel steps. Per step and direction:
           PE injects P[tok(chunk,step)] into PSUM via an identity matmul
           over a 32-slot-strided chunk AP (each PSUM bank sees exactly one
           start=True, its first writer, since start marks the whole bank
           pending-zero), then accumulates Whh @ h on top with fp8 DoubleRow
           matmuls (so the z-add never touches DVE); ACT runs sigmoid(i,f,g)
           on the critical path (g rows pre-scaled x2 so tanh(zg) =
           2*sig(2 zg)-1), sigmoid(o) off-path, and tanh(c); DVE does the
           bf16 gate algebra, the fp8 h for the next matmul, and the running
           max of h (warmup steps and the single tail-pad position excluded).

Recurring data is bf16 (DVE 2x mode) with fp8 h/Whh; PSUM accumulates fp32.
"""

import numpy as np
import ml_dtypes

import concourse.bass as bass
import concourse.mybir as mybir
from concourse import bacc
from concourse.tile import TileContext
from concourse.bass_utils import run_bass_kernel_spmd

F32 = mybir.dt.float32
BF16 = mybir.dt.bfloat16
FP8 = mybir.dt.float8e4
DR = mybir.MatmulPerfMode.DoubleRow
AF = mybir.ActivationFunctionType
ALU = mybir.AluOpType

S = 512
B = 64
E = 256
H = 256
NCORES = 8
BC = B // NCORES          # batch rows per core = 8
NT = 2 * S - 1            # stream steps per direction = 1023
KT = 2                    # k-tiles (contraction 256 = 2x128)
GT = 8                    # gate tiles (4H = 1024 = 8x128)

KC = 16                   # parallel chunks per direction
L = 64                    # chunk length (KC * L = 1024 >= NT)
W = 12                    # warmup steps per chunk
NP = L + W                # parallel steps
CC = KC * BC              # moving columns per direction = 128
Z = 8                     # zero-pad slots in front of P (>= ceil(W/2))
SLOTS = Z + S + 1         # P slots per (dir, gate-tile): pad + tokens + tail

TOKCOLS = S * BC          # 4096 x columns per k-tile in phase 1
CHUNK = 512               # moving cols per phase-1 matmul
NCHUNK = TOKCOLS // CHUNK # 8

# blob column layout (all bf16, 128 partitions):
XCOLS = KT * TOKCOLS                     # 8192
WIH_OFF = XCOLS                          # 2*8*2*128 = 4096  (d, g, k)
ID_OFF = WIH_OFF + 2 * GT * KT * 128     # 128
BIAS_OFF = ID_OFF + 128                  # 16
NBLOB = BIAS_OFF + 2 * GT
WHH8_COLS = 2 * GT * KT * 128            # fp8 whh, (d, g, k) blocks


def _fwd_slot(p):
    """P slot index for fwd stream position p (valid for p >= -2*Z)."""
    return Z + (p + 1) // 2


def _bwd_slot(p):
    """P slot index for bwd stream position p. Slot Z+S holds P_b[S-1] so the
    p == NT-1 cap falls out of the even-position formula."""
    if p % 2 == 0:
        return Z + p // 2 + 1
    return Z + (p - 1) // 2


def _build_program():
    import os
    debug = bool(os.environ.get("K_DEBUG"))
    nc = bacc.Bacc(None, target_bir_lowering=False)
    blob = nc.dram_tensor("blob", [128, NBLOB], BF16, kind="ExternalInput")
    whh8 = nc.dram_tensor("whh8", [128, WHH8_COLS], FP8, kind="ExternalInput")
    out = nc.dram_tensor("out", [128, 2 * KT * BC], F32, kind="ExternalOutput")
    if debug:
        dbg_p = nc.dram_tensor("dbg_p", [128, 2 * GT * SLOTS * BC], BF16, kind="ExternalOutput")
        dbg_s = nc.dram_tensor("dbg_s", [128, 4 * GT * CC], BF16, kind="ExternalOutput")
        dbg_h = nc.dram_tensor("dbg_h", [128, 16 * KT * CC], BF16, kind="ExternalOutput")
        dbg_z = nc.dram_tensor("dbg_z", [128, 4 * GT * CC], F32, kind="ExternalOutput")

    with TileContext(nc) as tc:
        with (
            tc.tile_pool(name="const", bufs=1) as const_pool,
            tc.tile_pool(name="pbuf", bufs=1) as p_pool,
            tc.tile_pool(name="sall", bufs=2) as sall_pool,
            tc.tile_pool(name="chain", bufs=2) as chain_pool,
            tc.tile_pool(name="state", bufs=2) as state_pool,
            tc.tile_pool(name="acc", bufs=1) as acc_pool,
            tc.tile_pool(name="zpsum", bufs=2, space="PSUM") as zpsum,
        ):
            blob_sb = const_pool.tile([128, NBLOB], BF16)
            # X is packed piece-major (4 pieces of 2 chunks, each piece holding
            # both k-tiles contiguously) so each DMA piece is one flat range;
            # weights/consts go first so phase-1 matmuls start early.
            nc.sync.dma_start(blob_sb[:, XCOLS:], blob[:, XCOLS:])
            PIECE = XCOLS // NCHUNK
            for piece in range(NCHUNK):
                cols = slice(piece * PIECE, (piece + 1) * PIECE)
                nc.sync.dma_start(blob_sb[:, cols], blob[:, cols])

            def x_ap(k, chk):
                off = chk * PIECE + k * CHUNK
                return blob_sb[:, off:off + CHUNK]

            def wih_ap(d, g, k):
                off = WIH_OFF + ((d * GT + g) * KT + k) * 128
                return blob_sb[:, off:off + 128]

            whh_sb = const_pool.tile([128, WHH8_COLS], FP8)
            nc.sync.dma_start(whh_sb[:], whh8[:])
            whh_view = whh_sb[:].rearrange("p (d g k m) -> p d g k m", d=2, g=GT, k=KT)

            ident = blob_sb[:, ID_OFF:ID_OFF + 128]

            # biases must be fp32 scalar-APs; upconvert once and pre-touch on
            # each engine that will use them as a tensor_scalar operand so the
            # dependency is already in that engine's vector clock (walrus
            # allows one sync-wait per compute instruction).
            bias_f32 = const_pool.tile([128, 2 * GT], F32)
            nc.vector.tensor_copy(bias_f32[:], blob_sb[:, BIAS_OFF:BIAS_OFF + 2 * GT])
            probe_v = const_pool.tile([128, 1], F32)
            nc.vector.tensor_copy(probe_v[:], bias_f32[:, 0:1])
            probe_a = const_pool.tile([128, 1], F32)
            nc.scalar.copy(probe_a[:], bias_f32[:, 0:1])

            def bias_ap(d, g):
                off = d * GT + g
                return bias_f32[:, off:off + 1]

            # P storage: (128, dir, gate-tile, slot, batch) bf16
            p_sb = p_pool.tile([128, 2 * GT * SLOTS * BC], BF16)
            p_view = p_sb[:].rearrange(
                "p (d g t b) -> p d g t b", d=2, g=GT, t=SLOTS, b=BC
            )

            # zero the warmup pad and the tail slot up front
            nc.vector.memset(p_view[:, :, :, 0:Z, :], 0)
            nc.vector.memset(p_view[:, :, :, Z + S, :], 0)

            # ---------------- phase 1: input projections ----------------
            # chunk-outer so compute on chunk c overlaps the DMA of chunk c+2
            for chk in range(NCHUNK):
                for d in range(2):
                    for g in range(GT):
                        zt = zpsum.tile([128, GT * CC], F32, tag=f"z{d}")
                        ps = zt[:, 0:CHUNK]
                        for k in range(KT):
                            nc.tensor.matmul(
                                ps,
                                wih_ap(d, g, k),
                                x_ap(k, chk),
                                start=(k == 0),
                                stop=(k == KT - 1),
                            )
                        dst = p_view[
                            :, d, g, Z + chk * (CHUNK // BC):Z + (chk + 1) * (CHUNK // BC), :
                        ]
                        if g % 2 == 0:
                            nc.scalar.activation(dst, ps, AF.Identity, bias=bias_ap(d, g))
                        else:
                            nc.vector.tensor_scalar(dst, ps, bias_ap(d, g), None, ALU.add)

            # bwd tail slot: stream position NT-1 maps to slot Z+S = P_b[S-1]
            nc.vector.tensor_copy(
                p_view[:, 1, :, Z + S, :], p_view[:, 1, :, Z + S - 1, :]
            )

            # ---------------- phase 2: chunk-parallel recurrence ----------------
            hmax_all = acc_pool.tile([128, 2, KT * CC], BF16, name="hmax")
            h_prev = [None, None]
            c_prev = [None, None]
            slot_of = [_fwd_slot, _bwd_slot]

            for t in range(NP):
                hb_step = None
                for d in range(2):
                    zt = zpsum.tile([128, GT * CC], F32, tag=f"z{d}", name=f"z{d}_{t}")
                    z5 = zt[:].rearrange(
                        "p (h g j b) -> p h g j b", h=2, g=GT // 2, j=KC, b=BC
                    )
                    # inject P. start=True marks the whole 2KB PSUM bank
                    # "pending zero", so each bank gets exactly ONE start=True
                    # (its first writer); later matmuls use start=False and
                    # the pending mechanism overwrites on first touch.
                    # The chunk stride is 32 slots, and slot(p0+L)-32 equals
                    # chunk 0's correct slot at every step except one warmup
                    # step per direction (the +1 rounding in the stream->token
                    # map sends the still-invalid position to token 0); that
                    # step splits chunk 0 into its own pad-slot matmul.
                    p0 = t - W
                    sl_j1 = slot_of[d](L + p0)          # chunk 1 slot
                    base = sl_j1 - 32                   # merged chunk-0 slot
                    merged = (base == slot_of[d](p0)) if p0 >= 0 else (base < Z)
                    for gh in range(2):
                        if merged:
                            rhs = p_view[
                                :, d, gh * 4:(gh + 1) * 4,
                                base:base + 32 * (KC - 1) + 1:32, :,
                            ]
                            nc.tensor.matmul(
                                z5[:, gh, :, :, :], ident, rhs,
                                start=True, stop=(t == 0),
                                skip_group_check=True,
                            )
                        else:
                            rhs_b = p_view[
                                :, d, gh * 4:(gh + 1) * 4,
                                sl_j1:sl_j1 + 32 * (KC - 2) + 1:32, :,
                            ]
                            nc.tensor.matmul(
                                z5[:, gh, :, 1:KC, :], ident, rhs_b,
                                start=True, stop=False, skip_group_check=True,
                            )
                            rhs_0 = p_view[:, d, gh * 4:(gh + 1) * 4, 0:1, :]
                            nc.tensor.matmul(
                                z5[:, gh, :, 0:1, :], ident, rhs_0,
                                start=False, stop=(t == 0), skip_group_check=True,
                            )
                    if t > 0:
                        h8 = h_prev[d]  # (128, KT, CC) fp8, k-subtile-major
                        # i/f/g tiles first so the sigmoid they feed can start
                        # before the o tiles (read by the off-path sigmoid) land
                        for g in range(GT):
                            nc.tensor.matmul(
                                zt[:, g * CC:(g + 1) * CC],
                                whh_view[:, d, g, :, :],
                                h8[:],
                                start=False,
                                stop=(g in (GT // 2 - 1, GT - 1)),
                                perf_mode=DR,
                                skip_group_check=True,
                            )

                    if debug and t < 2:
                        zc = acc_pool.tile([128, GT * CC], F32, name=f"zc{d}_{t}")
                        nc.vector.tensor_copy(zc[:], zt[:])
                        nc.sync.dma_start(
                            dbg_z[:, (t * 2 + d) * GT * CC:(t * 2 + d + 1) * GT * CC],
                            zc[:],
                        )
                    # gates: sigmoid over i,f,g (g rows pre-scaled x2) on the
                    # critical path; sigmoid over o separately (only h, late in
                    # the chain, needs it — keeps 25% of sigmoid off the path)
                    sall = sall_pool.tile([128, GT * CC], BF16, tag=f"sall{d}", name=f"sall{d}_{t}")
                    nc.scalar.activation(sall[:, 0:6 * CC], zt[:, 0:6 * CC], AF.Sigmoid)
                    nc.scalar.activation(sall[:, 6 * CC:], zt[:, 6 * CC:], AF.Sigmoid)
                    s_i = sall[:, 0:2 * CC]
                    s_f = sall[:, 2 * CC:4 * CC]
                    s_g = sall[:, 4 * CC:6 * CC]
                    s_o = sall[:, 6 * CC:8 * CC]

                    tg = chain_pool.tile([128, 2 * CC], BF16, tag=f"tg{d}", name=f"tg{d}_{t}")
                    nc.vector.tensor_scalar(tg[:], s_g, 2.0, -1.0, ALU.mult, ALU.add)
                    c_new = state_pool.tile([128, 2 * CC], BF16, tag=f"c{d}", name=f"c{d}_{t}")
                    if t == 0:
                        nc.vector.tensor_mul(c_new[:], s_i, tg[:])
                    else:
                        t1 = chain_pool.tile([128, 2 * CC], BF16, tag=f"t1{d}", name=f"t1{d}_{t}")
                        nc.vector.tensor_mul(t1[:], s_i, tg[:])
                        t2 = chain_pool.tile([128, 2 * CC], BF16, tag=f"t2{d}", name=f"t2{d}_{t}")
                        nc.vector.tensor_mul(t2[:], s_f, c_prev[d][:])
                        nc.vector.tensor_add(c_new[:], t1[:], t2[:])
                    th = chain_pool.tile([128, 2 * CC], BF16, tag=f"th{d}", name=f"th{d}_{t}")
                    nc.scalar.activation(th[:], c_new[:], AF.Tanh)
                    h_new = state_pool.tile([128, KT, CC], FP8, tag=f"h{d}", name=f"h{d}_{t}")
                    nc.vector.tensor_mul(
                        h_new[:].rearrange("p k c -> p (k c)"), s_o, th[:])
                    hb = None
                    if t >= W:
                        if hb_step is None:
                            hb_step = chain_pool.tile(
                                [128, 2, 2 * CC], BF16, tag="hball", name=f"hb_{t}")
                        hb = hb_step[:, d, :]
                        nc.vector.tensor_mul(hb, s_o, th[:])

                    DBG_TS = (12, 13, 20, 30, 40, 50, 60, 75)
                    if debug and t in DBG_TS and hb is not None:
                        ti = DBG_TS.index(t)
                        base = (ti * 2 + d) * KT * CC
                        nc.sync.dma_start(dbg_h[:, base:base + KT * CC], hb)

                    h_prev[d] = h_new
                    c_prev[d] = c_new

                # merged running max over both directions (off the critical
                # path, so waiting on the later direction's h is harmless)
                if t >= W:
                    if t == NP - 1:
                        # last step: chunk KC-1 sits on the single pad
                        # position past the stream end; exclude it.
                        hv = hb_step[:].rearrange(
                            "p d (k j b) -> p d k j b", k=KT, j=KC)
                        mv = hmax_all[:].rearrange(
                            "p d (k j b) -> p d k j b", k=KT, j=KC)
                        nc.vector.tensor_max(
                            mv[:, :, :, 0:KC - 1, :], mv[:, :, :, 0:KC - 1, :],
                            hv[:, :, :, 0:KC - 1, :],
                        )
                    elif t == W:
                        nc.vector.tensor_copy(hmax_all[:], hb_step[:])
                    else:
                        nc.vector.tensor_max(hmax_all[:], hmax_all[:], hb_step[:])

            # ---------------- final: fold chunks, emit ----------------
            out_sb = acc_pool.tile([128, 2 * KT * BC], F32)
            for d in range(2):
                m = hmax_all[:, d, :].rearrange("p (k j b) -> p k j b", k=KT, j=KC)
                for half in (8, 4, 2, 1):
                    nc.vector.tensor_max(
                        m[:, :, 0:half, :], m[:, :, 0:half, :],
                        m[:, :, half:2 * half, :],
                    )
                nc.vector.tensor_copy(
                    out_sb[:, d * KT * BC:(d + 1) * KT * BC],
                    m[:, :, 0, :],
                )
            nc.sync.dma_start(out[:], out_sb[:])
            if debug:
                nc.sync.dma_start(dbg_p[:], p_sb[:])

    nc.compile()
    return nc


def _pack_blob(X, weights):
    """Build per-core (128, NBLOB) bf16 blobs.

    g-gate rows are pre-scaled x2 so the kernel can evaluate
    tanh(zg) = 2*sigmoid(2*zg) - 1 with the single all-gates sigmoid.
    """
    bf = ml_dtypes.bfloat16
    img_common = np.zeros((128, NBLOB - XCOLS), np.float32)
    whh8img = np.empty((128, WHH8_COLS), ml_dtypes.float8_e4m3)

    for d, nm in enumerate("fb"):
        wih_p = weights[f"wih_{nm}"].astype(np.float32).copy()
        whh_p = weights[f"whh_{nm}"].astype(np.float32).copy()
        bias_p = (weights[f"bih_{nm}"] + weights[f"bhh_{nm}"]).astype(np.float32).copy()
        wih_p[2 * H:3 * H] *= 2.0
        whh_p[2 * H:3 * H] *= 2.0
        bias_p[2 * H:3 * H] *= 2.0
        for g in range(GT):
            for k in range(KT):
                blkT = wih_p[g * 128:(g + 1) * 128, k * 128:(k + 1) * 128].T
                off = WIH_OFF - XCOLS + ((d * GT + g) * KT + k) * 128
                img_common[:, off:off + 128] = blkT
                blkT = whh_p[g * 128:(g + 1) * 128, k * 128:(k + 1) * 128].T
                off = ((d * GT + g) * KT + k) * 128
                whh8img[:, off:off + 128] = blkT.astype(ml_dtypes.float8_e4m3)
            img_common[:, BIAS_OFF - XCOLS + d * GT + g] = bias_p[g * 128:(g + 1) * 128]
    img_common[:, ID_OFF - XCOLS:ID_OFF - XCOLS + 128] = np.eye(128, dtype=np.float32)

    Xt = np.ascontiguousarray(np.transpose(X, (2, 0, 1)))  # (E, S, B)
    piece_cols = TOKCOLS // NCHUNK
    blobs = []
    for c in range(NCORES):
        img = np.empty((128, NBLOB), np.float32)
        xc = Xt[:, :, c * BC:(c + 1) * BC].reshape(KT, 128, TOKCOLS)
        for piece in range(NCHUNK):
            for k in range(KT):
                off = piece * (XCOLS // NCHUNK) + k * piece_cols
                img[:, off:off + piece_cols] = \
                    xc[k][:, piece * piece_cols:(piece + 1) * piece_cols]
        img[:, XCOLS:] = img_common
        blobs.append(img.astype(bf))
    return blobs, whh8img


_PROGRAM_CACHE = {}


def _get_program():
    if "nc" not in _PROGRAM_CACHE:
        _PROGRAM_CACHE["nc"] = _build_program()
    return _PROGRAM_CACHE["nc"]


def _run(inputs, trace=False):
    X = np.asarray(inputs["inputs"], np.float32)
    blobs, whh8img = _pack_blob(X, inputs)
    nc = _get_program()
    in_maps = [{"blob": b, "whh8": whh8img} for b in blobs]
    res = run_bass_kernel_spmd(nc, in_maps, core_ids=list(range(NCORES)), trace=trace)
    # assemble (B, 2H): out[p, d*16 + k*8 + b] = h_d[dim 128k+p, batch b]
    emb = np.empty((B, 2 * H), np.float32)
    for c in range(NCORES):
        o = res.results[c]["out"]  # (128, 32)
        for d in range(2):
            for k in range(KT):
                blk = o[:, (d * KT + k) * BC:(d * KT + k + 1) * BC]  # (128, BC)
                emb[c * BC:(c + 1) * BC, d * H + k * 128:d * H + (k + 1) * 128] = blk.T
    return emb, res


def kernel(**inputs):
    emb, _ = _run(inputs, trace=False)
    return emb


# revision 27
# speedup vs baseline: 1.1199x; 1.1199x over previous
"""Trainium2 Bass kernel for the windowed bidirectional LSTM encoder.

Semantics (derived from the reference): each direction is a plain LSTM cell
chain over a token stream of length NT = 2S-1 (windows overlap, so tokens
repeat: fwd stream = x0,x1,x1,x2,x2,...,x511; bwd stream = x1,x0,x2,x1,...).
The output is the per-feature max of all NT hidden states of each direction,
concatenated: emb = [max_t h_f(t) | max_t h_b(t)] -> (B, 2H).

Distribution: 8 cores, each owns 8 batch rows and runs both directions.

Chunk-parallel recurrence: the LSTM map is contractive (forget gate < 0.82),
so the NT-step chain is split into K=16 chunks of L=64 steps processed in
parallel as extra "batch" columns. Each chunk is warmed up W=12 steps from a
zero state starting inside the previous chunk's range; warmed-up state error
is ~0.6^W ~ 2e-3. Chunk 0's warmup reads a zero-padded P region, which keeps
its state exactly (0,0) until its true start.

Per-core phases (GPSIMD compute is unavailable on this image, so everything
pointwise lives on DVE + ACT):
  phase 1: P[d,g,tok,b] = bf16(Wih_d @ x_tok + bias_d) for all 512 tokens
           (weights-stationary matmuls over piece-wise DMA'd X; bias folded
           into the PSUM->SBUF copies, which alternate ACT/DVE).
  phase 2: NP = L+W parallel steps. Per step and direction:
           PE injects P[tok(chunk,step)] into PSUM via an identity matmul
           over a 32-slot-strided chunk AP (each PSUM bank sees exactly one
           start=True, its first writer, since start marks the whole bank
           pending-zero), then accumulates Whh @ h on top with fp8 DoubleRow
           matmuls (so the z-add never touches DVE); ACT runs sigmoid(i,f,g)
           on the critical path (g rows pre-scaled x2 so tanh(zg) =
           2*sig(2 zg)-1), sigmoid(o) off-path, and tanh(c); DVE does the
           bf16 gate algebra, the fp8 h for the next matmul, and the running
           max of h (warmup steps and the single tail-pad position excluded).

Recurring data is bf16 (DVE 2x mode) with fp8 h/Whh; PSUM accumulates fp32.
"""

import numpy as np
import ml_dtypes

import concourse.bass as bass
import concourse.mybir as mybir
from concourse import bacc
from concourse.tile import TileContext
from concourse.bass_utils import run_bass_kernel_spmd

F32 = mybir.dt.float32
BF16 = mybir.dt.bfloat16
FP8 = mybir.dt.float8e4
DR = mybir.MatmulPerfMode.DoubleRow
AF = mybir.ActivationFunctionType
ALU = mybir.AluOpType

S = 512
B = 64
E = 256
H = 256
NCORES = 8
BC = B // NCORES          # batch rows per core = 8
NT = 2 * S - 1            # stream steps per direction = 1023
KT = 2                    # k-tiles (contraction 256 = 2x128)
GT = 8                    # gate tiles (4H = 1024 = 8x128)

KC = 16                   # parallel chunks per direction
L = 64                    # chunk length (KC * L = 1024 >= NT)
W = 11                    # warmup steps per chunk
NP = L + W                # parallel steps
CC = KC * BC              # moving columns per direction = 128
Z = 8                     # zero-pad slots in front of P (>= ceil(W/2))
SLOTS = Z + S + 1         # P slots per (dir, gate-tile): pad + tokens + tail

TOKCOLS = S * BC          # 4096 x columns per k-tile in phase 1
CHUNK = 512               # moving cols per phase-1 matmul
NCHUNK = TOKCOLS // CHUNK # 8

# blob column layout (all bf16, 128 partitions):
XCOLS = KT * TOKCOLS                     # 8192
WIH_OFF = XCOLS                          # 2*8*2*128 = 4096  (d, g, k)
ID_OFF = WIH_OFF + 2 * GT * KT * 128     # 128
BIAS_OFF = ID_OFF + 128                  # 16
NBLOB = BIAS_OFF + 2 * GT
WHH8_COLS = 2 * GT * KT * 128            # fp8 whh, (d, g, k) blocks


def _fwd_slot(p):
    """P slot index for fwd stream position p (valid for p >= -2*Z)."""
    return Z + (p + 1) // 2


def _bwd_slot(p):
    """P slot index for bwd stream position p. Slot Z+S holds P_b[S-1] so the
    p == NT-1 cap falls out of the even-position formula."""
    if p % 2 == 0:
        return Z + p // 2 + 1
    return Z + (p - 1) // 2


def _build_program():
    import os
    debug = bool(os.environ.get("K_DEBUG"))
    nc = bacc.Bacc(None, target_bir_lowering=False)
    blob = nc.dram_tensor("blob", [128, NBLOB], BF16, kind="ExternalInput")
    whh8 = nc.dram_tensor("whh8", [128, WHH8_COLS], FP8, kind="ExternalInput")
    out = nc.dram_tensor("out", [128, 2 * KT * BC], F32, kind="ExternalOutput")
    if debug:
        dbg_p = nc.dram_tensor("dbg_p", [128, 2 * GT * SLOTS * BC], BF16, kind="ExternalOutput")
        dbg_s = nc.dram_tensor("dbg_s", [128, 4 * GT * CC], BF16, kind="ExternalOutput")
        dbg_h = nc.dram_tensor("dbg_h", [128, 16 * KT * CC], BF16, kind="ExternalOutput")
        dbg_z = nc.dram_tensor("dbg_z", [128, 4 * GT * CC], F32, kind="ExternalOutput")

    with TileContext(nc) as tc:
        with (
            tc.tile_pool(name="const", bufs=1) as const_pool,
            tc.tile_pool(name="pbuf", bufs=1) as p_pool,
            tc.tile_pool(name="sall", bufs=2) as sall_pool,
            tc.tile_pool(name="chain", bufs=2) as chain_pool,
            tc.tile_pool(name="state", bufs=2) as state_pool,
            tc.tile_pool(name="acc", bufs=1) as acc_pool,
            tc.tile_pool(name="zpsum", bufs=2, space="PSUM") as zpsum,
        ):
            blob_sb = const_pool.tile([128, NBLOB], BF16)
            # X is packed piece-major (4 pieces of 2 chunks, each piece holding
            # both k-tiles contiguously) so each DMA piece is one flat range;
            # weights/consts go first so phase-1 matmuls start early.
            nc.sync.dma_start(blob_sb[:, XCOLS:], blob[:, XCOLS:])
            PIECE = XCOLS // 4
            for piece in range(4):
                cols = slice(piece * PIECE, (piece + 1) * PIECE)
                nc.sync.dma_start(blob_sb[:, cols], blob[:, cols])

            def x_ap(k, chk):
                off = (chk // 2) * PIECE + k * (PIECE // 2) + (chk % 2) * CHUNK
                return blob_sb[:, off:off + CHUNK]

            def wih_ap(d, g, k):
                off = WIH_OFF + ((d * GT + g) * KT + k) * 128
                return blob_sb[:, off:off + 128]

            whh_sb = const_pool.tile([128, WHH8_COLS], FP8)
            nc.sync.dma_start(whh_sb[:], whh8[:])
            whh_view = whh_sb[:].rearrange("p (d g k m) -> p d g k m", d=2, g=GT, k=KT)

            ident = blob_sb[:, ID_OFF:ID_OFF + 128]

            # biases must be fp32 scalar-APs; upconvert once and pre-touch on
            # each engine that will use them as a tensor_scalar operand so the
            # dependency is already in that engine's vector clock (walrus
            # allows one sync-wait per compute instruction).
            bias_f32 = const_pool.tile([128, 2 * GT], F32)
            nc.vector.tensor_copy(bias_f32[:], blob_sb[:, BIAS_OFF:BIAS_OFF + 2 * GT])
            probe_v = const_pool.tile([128, 1], F32)
            nc.vector.tensor_copy(probe_v[:], bias_f32[:, 0:1])
            probe_a = const_pool.tile([128, 1], F32)
            nc.scalar.copy(probe_a[:], bias_f32[:, 0:1])

            def bias_ap(d, g):
                off = d * GT + g
                return bias_f32[:, off:off + 1]

            # P storage: (128, dir, gate-tile, slot, batch) bf16
            p_sb = p_pool.tile([128, 2 * GT * SLOTS * BC], BF16)
            p_view = p_sb[:].rearrange(
                "p (d g t b) -> p d g t b", d=2, g=GT, t=SLOTS, b=BC
            )

            # zero the warmup pad and the tail slot up front
            nc.vector.memset(p_view[:, :, :, 0:Z, :], 0)
            nc.vector.memset(p_view[:, :, :, Z + S, :], 0)

            # ---------------- phase 1: input projections ----------------
            # chunk-outer so compute on chunk c overlaps the DMA of chunk c+2
            for chk in range(NCHUNK):
                for d in range(2):
                    for g in range(GT):
                        zt = zpsum.tile([128, GT * CC], F32, tag=f"z{d}")
                        ps = zt[:, 0:CHUNK]
                        for k in range(KT):
                            nc.tensor.matmul(
                                ps,
                                wih_ap(d, g, k),
                                x_ap(k, chk),
                                start=(k == 0),
                                stop=(k == KT - 1),
                            )
                        dst = p_view[
                            :, d, g, Z + chk * (CHUNK // BC):Z + (chk + 1) * (CHUNK // BC), :
                        ]
                        if g % 2 == 0:
                            nc.scalar.activation(dst, ps, AF.Identity, bias=bias_ap(d, g))
                        else:
                            nc.vector.tensor_scalar(dst, ps, bias_ap(d, g), None, ALU.add)

            # bwd tail slot: stream position NT-1 maps to slot Z+S = P_b[S-1]
            nc.vector.tensor_copy(
                p_view[:, 1, :, Z + S, :], p_view[:, 1, :, Z + S - 1, :]
            )

            # ---------------- phase 2: chunk-parallel recurrence ----------------
            hmax = [acc_pool.tile([128, KT * CC], BF16, name=f"hmax{d}") for d in range(2)]
            h_prev = [None, None]
            c_prev = [None, None]
            slot_of = [_fwd_slot, _bwd_slot]

            for t in range(NP):
                deferred = []
                for d in range(2):
                    zt = zpsum.tile([128, GT * CC], F32, tag=f"z{d}", name=f"z{d}_{t}")
                    z5 = zt[:].rearrange(
                        "p (h g j b) -> p h g j b", h=2, g=GT // 2, j=KC, b=BC
                    )
                    # inject P. start=True marks the whole 2KB PSUM bank
                    # "pending zero", so each bank gets exactly ONE start=True
                    # (its first writer); later matmuls use start=False and
                    # the pending mechanism overwrites on first touch.
                    # The chunk stride is 32 slots, and slot(p0+L)-32 equals
                    # chunk 0's correct slot at every step except one warmup
                    # step per direction (the +1 rounding in the stream->token
                    # map sends the still-invalid position to token 0); that
                    # step splits chunk 0 into its own pad-slot matmul.
                    p0 = t - W
                    sl_j1 = slot_of[d](L + p0)          # chunk 1 slot
                    base = sl_j1 - 32                   # merged chunk-0 slot
                    merged = (base == slot_of[d](p0)) if p0 >= 0 else (base < Z)
                    for gh in range(2):
                        if merged:
                            rhs = p_view[
                                :, d, gh * 4:(gh + 1) * 4,
                                base:base + 32 * (KC - 1) + 1:32, :,
                            ]
                            nc.tensor.matmul(
                                z5[:, gh, :, :, :], ident, rhs,
                                start=True, stop=(t == 0),
                                skip_group_check=True,
                            )
                        else:
                            rhs_b = p_view[
                                :, d, gh * 4:(gh + 1) * 4,
                                sl_j1:sl_j1 + 32 * (KC - 2) + 1:32, :,
                            ]
                            nc.tensor.matmul(
                                z5[:, gh, :, 1:KC, :], ident, rhs_b,
                                start=True, stop=False, skip_group_check=True,
                            )
                            rhs_0 = p_view[:, d, gh * 4:(gh + 1) * 4, 0:1, :]
                            nc.tensor.matmul(
                                z5[:, gh, :, 0:1, :], ident, rhs_0,
                                start=False, stop=(t == 0), skip_group_check=True,
                            )
                    if t > 0:
                        h8 = h_prev[d]  # (128, KT, CC) fp8, k-subtile-major
                        # i/f/g tiles first so the sigmoid they feed can start
                        # before the o tiles (read by the off-path sigmoid) land
                        for g in range(GT):
                            nc.tensor.matmul(
                                zt[:, g * CC:(g + 1) * CC],
                                whh_view[:, d, g, :, :],
                                h8[:],
                                start=False,
                                stop=(g in (GT // 2 - 1, GT - 1)),
                                perf_mode=DR,
                                skip_group_check=True,
                            )

                    if debug and t < 2:
                        zc = acc_pool.tile([128, GT * CC], F32, name=f"zc{d}_{t}")
                        nc.vector.tensor_copy(zc[:], zt[:])
                        nc.sync.dma_start(
                            dbg_z[:, (t * 2 + d) * GT * CC:(t * 2 + d + 1) * GT * CC],
                            zc[:],
                        )
                    # gates: sigmoid over i,f,g (g rows pre-scaled x2) on the
                    # critical path; sigmoid over o separately (only h, late in
                    # the chain, needs it — keeps 25% of sigmoid off the path)
                    sall = sall_pool.tile([128, GT * CC], BF16, tag=f"sall{d}", name=f"sall{d}_{t}")
                    nc.scalar.activation(sall[:, 0:6 * CC], zt[:, 0:6 * CC], AF.Sigmoid)
                    nc.scalar.activation(sall[:, 6 * CC:], zt[:, 6 * CC:], AF.Sigmoid)
                    s_i = sall[:, 0:2 * CC]
                    s_f = sall[:, 2 * CC:4 * CC]
                    s_g = sall[:, 4 * CC:6 * CC]
                    s_o = sall[:, 6 * CC:8 * CC]

                    tg = chain_pool.tile([128, 2 * CC], BF16, tag=f"tg{d}", name=f"tg{d}_{t}")
                    nc.vector.tensor_scalar(tg[:], s_g, 2.0, -1.0, ALU.mult, ALU.add)
                    c_new = state_pool.tile([128, 2 * CC], BF16, tag=f"c{d}", name=f"c{d}_{t}")
                    if t == 0:
                        nc.vector.tensor_mul(c_new[:], s_i, tg[:])
                    else:
                        t1 = chain_pool.tile([128, 2 * CC], BF16, tag=f"t1{d}", name=f"t1{d}_{t}")
                        nc.vector.tensor_mul(t1[:], s_i, tg[:])
                        t2 = chain_pool.tile([128, 2 * CC], BF16, tag=f"t2{d}", name=f"t2{d}_{t}")
                        nc.vector.tensor_mul(t2[:], s_f, c_prev[d][:])
                        nc.vector.tensor_add(c_new[:], t1[:], t2[:])
                    th = chain_pool.tile([128, 2 * CC], BF16, tag=f"th{d}", name=f"th{d}_{t}")
                    nc.scalar.activation(th[:], c_new[:], AF.Tanh)
                    h_new = state_pool.tile([128, KT, CC], FP8, tag=f"h{d}", name=f"h{d}_{t}")
                    nc.vector.tensor_mul(
                        h_new[:].rearrange("p k c -> p (k c)"), s_o, th[:])
                    # defer the (off-path) h-for-max mul and running max to
                    # after both directions' on-path ops so they don't sit
                    # ahead of the other direction's chain in the in-order
                    # DVE queue; tiles stay per-direction (no coupling).
                    def _deferred_max(d=d, t=t, s_o=s_o, th=th):
                        hb = chain_pool.tile([128, 2 * CC], BF16, tag=f"hb{d}", name=f"hb{d}_{t}")
                        nc.vector.tensor_mul(hb[:], s_o, th[:])
                        if t == NP - 1:
                            # last step: chunk KC-1 sits on the single pad
                            # position past the stream end; exclude it.
                            hv = hb[:].rearrange("p (k j b) -> p k j b", k=KT, j=KC)
                            mv = hmax[d][:].rearrange("p (k j b) -> p k j b", k=KT, j=KC)
                            nc.vector.tensor_max(
                                mv[:, :, 0:KC - 1, :], mv[:, :, 0:KC - 1, :],
                                hv[:, :, 0:KC - 1, :],
                            )
                        elif t == W:
                            nc.vector.tensor_copy(hmax[d][:], hb[:])
                        else:
                            nc.vector.tensor_max(hmax[d][:], hmax[d][:], hb[:])
                        return hb
                    if t >= W:
                        deferred.append(_deferred_max)

                    h_prev[d] = h_new
                    c_prev[d] = c_new

                for fn in deferred:
                    fn()

            # ---------------- final: fold chunks, emit ----------------
            out_sb = acc_pool.tile([128, 2 * KT * BC], F32)
            for d in range(2):
                m = hmax[d][:].rearrange("p (k j b) -> p k j b", k=KT, j=KC)
                for half in (8, 4, 2, 1):
                    nc.vector.tensor_max(
                        m[:, :, 0:half, :], m[:, :, 0:half, :],
                        m[:, :, half:2 * half, :],
                    )
                nc.vector.tensor_copy(
                    out_sb[:, d * KT * BC:(d + 1) * KT * BC],
                    m[:, :, 0, :],
                )
            nc.sync.dma_start(out[:], out_sb[:])
            if debug:
                nc.sync.dma_start(dbg_p[:], p_sb[:])

    nc.compile()
    return nc


def _pack_blob(X, weights):
    """Build per-core (128, NBLOB) bf16 blobs.

    g-gate rows are pre-scaled x2 so the kernel can evaluate
    tanh(zg) = 2*sigmoid(2*zg) - 1 with the single all-gates sigmoid.
    """
    bf = ml_dtypes.bfloat16
    img_common = np.zeros((128, NBLOB - XCOLS), np.float32)
    whh8img = np.empty((128, WHH8_COLS), ml_dtypes.float8_e4m3)

    for d, nm in enumerate("fb"):
        wih_p = weights[f"wih_{nm}"].astype(np.float32).copy()
        whh_p = weights[f"whh_{nm}"].astype(np.float32).copy()
        bias_p = (weights[f"bih_{nm}"] + weights[f"bhh_{nm}"]).astype(np.float32).copy()
        wih_p[2 * H:3 * H] *= 2.0
        whh_p[2 * H:3 * H] *= 2.0
        bias_p[2 * H:3 * H] *= 2.0
        for g in range(GT):
            for k in range(KT):
                blkT = wih_p[g * 128:(g + 1) * 128, k * 128:(k + 1) * 128].T
                off = WIH_OFF - XCOLS + ((d * GT + g) * KT + k) * 128
                img_common[:, off:off + 128] = blkT
                blkT = whh_p[g * 128:(g + 1) * 128, k * 128:(k + 1) * 128].T
                off = ((d * GT + g) * KT + k) * 128
                whh8img[:, off:off + 128] = blkT.astype(ml_dtypes.float8_e4m3)
            img_common[:, BIAS_OFF - XCOLS + d * GT + g] = bias_p[g * 128:(g + 1) * 128]
    img_common[:, ID_OFF - XCOLS:ID_OFF - XCOLS + 128] = np.eye(128, dtype=np.float32)

    Xt = np.ascontiguousarray(np.transpose(X, (2, 0, 1)))  # (E, S, B)
    piece_cols = TOKCOLS // 4
    blobs = []
    for c in range(NCORES):
        img = np.empty((128, NBLOB), np.float32)
        xc = Xt[:, :, c * BC:(c + 1) * BC].reshape(KT, 128, TOKCOLS)
        for piece in range(4):
            for k in range(KT):
                off = piece * (XCOLS // 4) + k * piece_cols
                img[:, off:off + piece_cols] = \
                    xc[k][:, piece * piece_cols:(piece + 1) * piece_cols]
        img[:, XCOLS:] = img_common
        blobs.append(img.astype(bf))
    return blobs, whh8img


_PROGRAM_CACHE = {}


def _get_program():
    if "nc" not in _PROGRAM_CACHE:
        _PROGRAM_CACHE["nc"] = _build_program()
    return _PROGRAM_CACHE["nc"]


def _run(inputs, trace=False):
    X = np.asarray(inputs["inputs"], np.float32)
    blobs, whh8img = _pack_blob(X, inputs)
    nc = _get_program()
    in_maps = [{"blob": b, "whh8": whh8img} for b in blobs]
    res = run_bass_kernel_spmd(nc, in_maps, core_ids=list(range(NCORES)), trace=trace)
    # assemble (B, 2H): out[p, d*16 + k*8 + b] = h_d[dim 128k+p, batch b]
    emb = np.empty((B, 2 * H), np.float32)
    for c in range(NCORES):
        o = res.results[c]["out"]  # (128, 32)
        for d in range(2):
            for k in range(KT):
                blk = o[:, (d * KT + k) * BC:(d * KT + k + 1) * BC]  # (128, BC)
                emb[c * BC:(c + 1) * BC, d * H + k * 128:d * H + (k + 1) * 128] = blk.T
    return emb, res


def kernel(**inputs):
    emb, _ = _run(inputs, trace=False)
    return emb


# revision 28
# speedup vs baseline: 1.1445x; 1.0220x over previous
"""Trainium2 Bass kernel for the windowed bidirectional LSTM encoder.

Semantics (derived from the reference): each direction is a plain LSTM cell
chain over a token stream of length NT = 2S-1 (windows overlap, so tokens
repeat: fwd stream = x0,x1,x1,x2,x2,...,x511; bwd stream = x1,x0,x2,x1,...).
The output is the per-feature max of all NT hidden states of each direction,
concatenated: emb = [max_t h_f(t) | max_t h_b(t)] -> (B, 2H).

Distribution: 8 cores, each owns 8 batch rows and runs both directions.

Chunk-parallel recurrence: the LSTM map is contractive (forget gate < 0.82),
so the NT-step chain is split into K=16 chunks of L=64 steps processed in
parallel as extra "batch" columns. Each chunk is warmed up W=12 steps from a
zero state starting inside the previous chunk's range; warmed-up state error
is ~0.6^W ~ 3e-3. Chunk 0's warmup reads a zero-padded P region, which keeps
its state exactly (0,0) until its true start.

Per-core phases (GPSIMD compute is unavailable on this image, so everything
pointwise lives on DVE + ACT):
  phase 1: P[d,g,tok,b] = bf16(Wih_d @ x_tok + bias_d) for all 512 tokens
           (weights-stationary matmuls over piece-wise DMA'd X; bias folded
           into the PSUM->SBUF copies, which alternate ACT/DVE).
  phase 2: NP = L+W parallel steps. Per step and direction:
           PE injects P[tok(chunk,step)] into PSUM via an identity matmul
           over a 32-slot-strided chunk AP (each PSUM bank sees exactly one
           start=True, its first writer, since start marks the whole bank
           pending-zero), then accumulates Whh @ h on top with fp8 DoubleRow
           matmuls (so the z-add never touches DVE); ACT runs sigmoid(i,f,g)
           on the critical path (g rows pre-scaled x2 so tanh(zg) =
           2*sig(2 zg)-1), sigmoid(o) off-path, and tanh(c); DVE does the
           bf16 gate algebra, the fp8 h for the next matmul, and the running
           max of h (warmup steps and the single tail-pad position excluded).

Recurring data is bf16 (DVE 2x mode) with fp8 h/Whh; PSUM accumulates fp32.
"""

import numpy as np
import ml_dtypes

import concourse.bass as bass
import concourse.mybir as mybir
from concourse import bacc
from concourse.tile import TileContext
from concourse.bass_utils import run_bass_kernel_spmd

F32 = mybir.dt.float32
BF16 = mybir.dt.bfloat16
FP8 = mybir.dt.float8e4
DR = mybir.MatmulPerfMode.DoubleRow
AF = mybir.ActivationFunctionType
ALU = mybir.AluOpType

S = 512
B = 64
E = 256
H = 256
NCORES = 8
BC = B // NCORES          # batch rows per core = 8
NT = 2 * S - 1            # stream steps per direction = 1023
KT = 2                    # k-tiles (contraction 256 = 2x128)
GT = 8                    # gate tiles (4H = 1024 = 8x128)

KC = 16                   # parallel chunks per direction
L = 64                    # chunk length (KC * L = 1024 >= NT)
W = 10                    # warmup steps per chunk
NP = L + W                # parallel steps
CC = KC * BC              # moving columns per direction = 128
Z = 8                     # zero-pad slots in front of P (>= ceil(W/2))
SLOTS = Z + S + 1         # P slots per (dir, gate-tile): pad + tokens + tail

TOKCOLS = S * BC          # 4096 x columns per k-tile in phase 1
CHUNK = 512               # moving cols per phase-1 matmul
NCHUNK = TOKCOLS // CHUNK # 8

# blob column layout (all bf16, 128 partitions):
XCOLS = KT * TOKCOLS                     # 8192
WIH_OFF = XCOLS                          # 2*8*2*128 = 4096  (d, g, k)
ID_OFF = WIH_OFF + 2 * GT * KT * 128     # 128
BIAS_OFF = ID_OFF + 128                  # 16
NBLOB = BIAS_OFF + 2 * GT
WHH8_COLS = 2 * GT * KT * 128            # fp8 whh, (d, g, k) blocks


def _fwd_slot(p):
    """P slot index for fwd stream position p (valid for p >= -2*Z)."""
    return Z + (p + 1) // 2


def _bwd_slot(p):
    """P slot index for bwd stream position p. Slot Z+S holds P_b[S-1] so the
    p == NT-1 cap falls out of the even-position formula."""
    if p % 2 == 0:
        return Z + p // 2 + 1
    return Z + (p - 1) // 2


def _build_program():
    import os
    debug = bool(os.environ.get("K_DEBUG"))
    nc = bacc.Bacc(None, target_bir_lowering=False)
    blob = nc.dram_tensor("blob", [128, NBLOB], BF16, kind="ExternalInput")
    whh8 = nc.dram_tensor("whh8", [128, WHH8_COLS], FP8, kind="ExternalInput")
    out = nc.dram_tensor("out", [128, 2 * KT * BC], F32, kind="ExternalOutput")
    if debug:
        dbg_p = nc.dram_tensor("dbg_p", [128, 2 * GT * SLOTS * BC], BF16, kind="ExternalOutput")
        dbg_s = nc.dram_tensor("dbg_s", [128, 4 * GT * CC], BF16, kind="ExternalOutput")
        dbg_h = nc.dram_tensor("dbg_h", [128, 16 * KT * CC], BF16, kind="ExternalOutput")
        dbg_z = nc.dram_tensor("dbg_z", [128, 4 * GT * CC], F32, kind="ExternalOutput")

    with TileContext(nc) as tc:
        with (
            tc.tile_pool(name="const", bufs=1) as const_pool,
            tc.tile_pool(name="pbuf", bufs=1) as p_pool,
            tc.tile_pool(name="sall", bufs=2) as sall_pool,
            tc.tile_pool(name="chain", bufs=2) as chain_pool,
            tc.tile_pool(name="state", bufs=2) as state_pool,
            tc.tile_pool(name="acc", bufs=1) as acc_pool,
            tc.tile_pool(name="zpsum", bufs=2, space="PSUM") as zpsum,
        ):
            blob_sb = const_pool.tile([128, NBLOB], BF16)
            # X is packed piece-major (4 pieces of 2 chunks, each piece holding
            # both k-tiles contiguously) so each DMA piece is one flat range;
            # weights/consts go first so phase-1 matmuls start early.
            nc.sync.dma_start(blob_sb[:, XCOLS:], blob[:, XCOLS:])
            PIECE = XCOLS // 4
            for piece in range(4):
                cols = slice(piece * PIECE, (piece + 1) * PIECE)
                nc.sync.dma_start(blob_sb[:, cols], blob[:, cols])

            def x_ap(k, chk):
                off = (chk // 2) * PIECE + k * (PIECE // 2) + (chk % 2) * CHUNK
                return blob_sb[:, off:off + CHUNK]

            def wih_ap(d, g, k):
                off = WIH_OFF + ((d * GT + g) * KT + k) * 128
                return blob_sb[:, off:off + 128]

            whh_sb = const_pool.tile([128, WHH8_COLS], FP8)
            nc.sync.dma_start(whh_sb[:], whh8[:])
            whh_view = whh_sb[:].rearrange("p (d g k m) -> p d g k m", d=2, g=GT, k=KT)

            ident = blob_sb[:, ID_OFF:ID_OFF + 128]

            # biases must be fp32 scalar-APs; upconvert once and pre-touch on
            # each engine that will use them as a tensor_scalar operand so the
            # dependency is already in that engine's vector clock (walrus
            # allows one sync-wait per compute instruction).
            bias_f32 = const_pool.tile([128, 2 * GT], F32)
            nc.vector.tensor_copy(bias_f32[:], blob_sb[:, BIAS_OFF:BIAS_OFF + 2 * GT])
            probe_v = const_pool.tile([128, 1], F32)
            nc.vector.tensor_copy(probe_v[:], bias_f32[:, 0:1])
            probe_a = const_pool.tile([128, 1], F32)
            nc.scalar.copy(probe_a[:], bias_f32[:, 0:1])

            def bias_ap(d, g):
                off = d * GT + g
                return bias_f32[:, off:off + 1]

            # P storage: (128, dir, gate-tile, slot, batch) bf16
            p_sb = p_pool.tile([128, 2 * GT * SLOTS * BC], BF16)
            p_view = p_sb[:].rearrange(
                "p (d g t b) -> p d g t b", d=2, g=GT, t=SLOTS, b=BC
            )

            # zero the warmup pad and the tail slot up front
            nc.vector.memset(p_view[:, :, :, 0:Z, :], 0)
            nc.vector.memset(p_view[:, :, :, Z + S, :], 0)

            # ---------------- phase 1: input projections ----------------
            # chunk-outer so compute on chunk c overlaps the DMA of chunk c+2
            for chk in range(NCHUNK):
                for d in range(2):
                    for g in range(GT):
                        zt = zpsum.tile([128, GT * CC], F32, tag=f"z{d}")
                        ps = zt[:, 0:CHUNK]
                        for k in range(KT):
                            nc.tensor.matmul(
                                ps,
                                wih_ap(d, g, k),
                                x_ap(k, chk),
                                start=(k == 0),
                                stop=(k == KT - 1),
                            )
                        dst = p_view[
                            :, d, g, Z + chk * (CHUNK // BC):Z + (chk + 1) * (CHUNK // BC), :
                        ]
                        if g % 2 == 0:
                            nc.scalar.activation(dst, ps, AF.Identity, bias=bias_ap(d, g))
                        else:
                            nc.vector.tensor_scalar(dst, ps, bias_ap(d, g), None, ALU.add)

            # bwd tail slot: stream position NT-1 maps to slot Z+S = P_b[S-1]
            nc.vector.tensor_copy(
                p_view[:, 1, :, Z + S, :], p_view[:, 1, :, Z + S - 1, :]
            )

            # ---------------- phase 2: chunk-parallel recurrence ----------------
            hmax = [acc_pool.tile([128, KT * CC], BF16, name=f"hmax{d}") for d in range(2)]
            h_prev = [None, None]
            c_prev = [None, None]
            slot_of = [_fwd_slot, _bwd_slot]

            for t in range(NP):
                deferred = []
                for d in range(2):
                    zt = zpsum.tile([128, GT * CC], F32, tag=f"z{d}", name=f"z{d}_{t}")
                    z5 = zt[:].rearrange(
                        "p (h g j b) -> p h g j b", h=2, g=GT // 2, j=KC, b=BC
                    )
                    # inject P. start=True marks the whole 2KB PSUM bank
                    # "pending zero", so each bank gets exactly ONE start=True
                    # (its first writer); later matmuls use start=False and
                    # the pending mechanism overwrites on first touch.
                    # The chunk stride is 32 slots, and slot(p0+L)-32 equals
                    # chunk 0's correct slot at every step except one warmup
                    # step per direction (the +1 rounding in the stream->token
                    # map sends the still-invalid position to token 0); that
                    # step splits chunk 0 into its own pad-slot matmul.
                    p0 = t - W
                    sl_j1 = slot_of[d](L + p0)          # chunk 1 slot
                    base = sl_j1 - 32                   # merged chunk-0 slot
                    merged = (base == slot_of[d](p0)) if p0 >= 0 else (base < Z)
                    for gh in range(2):
                        if merged:
                            rhs = p_view[
                                :, d, gh * 4:(gh + 1) * 4,
                                base:base + 32 * (KC - 1) + 1:32, :,
                            ]
                            nc.tensor.matmul(
                                z5[:, gh, :, :, :], ident, rhs,
                                start=True, stop=(t == 0),
                                skip_group_check=True,
                            )
                        else:
                            rhs_b = p_view[
                                :, d, gh * 4:(gh + 1) * 4,
                                sl_j1:sl_j1 + 32 * (KC - 2) + 1:32, :,
                            ]
                            nc.tensor.matmul(
                                z5[:, gh, :, 1:KC, :], ident, rhs_b,
                                start=True, stop=False, skip_group_check=True,
                            )
                            rhs_0 = p_view[:, d, gh * 4:(gh + 1) * 4, 0:1, :]
                            nc.tensor.matmul(
                                z5[:, gh, :, 0:1, :], ident, rhs_0,
                                start=False, stop=(t == 0), skip_group_check=True,
                            )
                    if t > 0:
                        h8 = h_prev[d]  # (128, KT, CC) fp8, k-subtile-major
                        # i/f/g tiles first so the sigmoid they feed can start
                        # before the o tiles (read by the off-path sigmoid) land
                        for g in range(GT):
                            nc.tensor.matmul(
                                zt[:, g * CC:(g + 1) * CC],
                                whh_view[:, d, g, :, :],
                                h8[:],
                                start=False,
                                stop=(g in (GT // 2 - 1, GT - 1)),
                                perf_mode=DR,
                                skip_group_check=True,
                            )

                    if debug and t < 2:
                        zc = acc_pool.tile([128, GT * CC], F32, name=f"zc{d}_{t}")
                        nc.vector.tensor_copy(zc[:], zt[:])
                        nc.sync.dma_start(
                            dbg_z[:, (t * 2 + d) * GT * CC:(t * 2 + d + 1) * GT * CC],
                            zc[:],
                        )
                    # gates: sigmoid over i,f,g (g rows pre-scaled x2) on the
                    # critical path; sigmoid over o separately (only h, late in
                    # the chain, needs it — keeps 25% of sigmoid off the path)
                    sall = sall_pool.tile([128, GT * CC], BF16, tag=f"sall{d}", name=f"sall{d}_{t}")
                    nc.scalar.activation(sall[:, 0:6 * CC], zt[:, 0:6 * CC], AF.Sigmoid)
                    nc.scalar.activation(sall[:, 6 * CC:], zt[:, 6 * CC:], AF.Sigmoid)
                    s_i = sall[:, 0:2 * CC]
                    s_f = sall[:, 2 * CC:4 * CC]
                    s_g = sall[:, 4 * CC:6 * CC]
                    s_o = sall[:, 6 * CC:8 * CC]

                    tg = chain_pool.tile([128, 2 * CC], BF16, tag=f"tg{d}", name=f"tg{d}_{t}")
                    nc.vector.tensor_scalar(tg[:], s_g, 2.0, -1.0, ALU.mult, ALU.add)
                    c_new = state_pool.tile([128, 2 * CC], BF16, tag=f"c{d}", name=f"c{d}_{t}")
                    if t == 0:
                        nc.vector.tensor_mul(c_new[:], s_i, tg[:])
                    else:
                        t1 = chain_pool.tile([128, 2 * CC], BF16, tag=f"t1{d}", name=f"t1{d}_{t}")
                        nc.vector.tensor_mul(t1[:], s_i, tg[:])
                        t2 = chain_pool.tile([128, 2 * CC], BF16, tag=f"t2{d}", name=f"t2{d}_{t}")
                        nc.vector.tensor_mul(t2[:], s_f, c_prev[d][:])
                        nc.vector.tensor_add(c_new[:], t1[:], t2[:])
                    th = chain_pool.tile([128, 2 * CC], BF16, tag=f"th{d}", name=f"th{d}_{t}")
                    nc.scalar.activation(th[:], c_new[:], AF.Tanh)
                    h_new = state_pool.tile([128, KT, CC], FP8, tag=f"h{d}", name=f"h{d}_{t}")
                    nc.vector.tensor_mul(
                        h_new[:].rearrange("p k c -> p (k c)"), s_o, th[:])
                    # defer the (off-path) h-for-max mul and running max to
                    # after both directions' on-path ops so they don't sit
                    # ahead of the other direction's chain in the in-order
                    # DVE queue; tiles stay per-direction (no coupling).
                    def _deferred_max(d=d, t=t, s_o=s_o, th=th):
                        hb = chain_pool.tile([128, 2 * CC], BF16, tag=f"hb{d}", name=f"hb{d}_{t}")
                        nc.vector.tensor_mul(hb[:], s_o, th[:])
                        if t == NP - 1:
                            # last step: chunk KC-1 sits on the single pad
                            # position past the stream end; exclude it.
                            hv = hb[:].rearrange("p (k j b) -> p k j b", k=KT, j=KC)
                            mv = hmax[d][:].rearrange("p (k j b) -> p k j b", k=KT, j=KC)
                            nc.vector.tensor_max(
                                mv[:, :, 0:KC - 1, :], mv[:, :, 0:KC - 1, :],
                                hv[:, :, 0:KC - 1, :],
                            )
                        elif t == W:
                            nc.vector.tensor_copy(hmax[d][:], hb[:])
                        else:
                            nc.vector.tensor_max(hmax[d][:], hmax[d][:], hb[:])
                        return hb
                    if t >= W:
                        deferred.append(_deferred_max)

                    h_prev[d] = h_new
                    c_prev[d] = c_new

                for fn in deferred:
                    fn()

            # ---------------- final: fold chunks, emit ----------------
            out_sb = acc_pool.tile([128, 2 * KT * BC], F32)
            for d in range(2):
                m = hmax[d][:].rearrange("p (k j b) -> p k j b", k=KT, j=KC)
                for half in (8, 4, 2, 1):
                    nc.vector.tensor_max(
                        m[:, :, 0:half, :], m[:, :, 0:half, :],
                        m[:, :, half:2 * half, :],
                    )
                nc.vector.tensor_copy(
                    out_sb[:, d * KT * BC:(d + 1) * KT * BC],
                    m[:, :, 0, :],
                )
            nc.sync.dma_start(out[:], out_sb[:])
            if debug:
                nc.sync.dma_start(dbg_p[:], p_sb[:])

    nc.compile()
    return nc


def _pack_blob(X, weights):
    """Build per-core (128, NBLOB) bf16 blobs.

    g-gate rows are pre-scaled x2 so the kernel can evaluate
    tanh(zg) = 2*sigmoid(2*zg) - 1 with the single all-gates sigmoid.
    """
    bf = ml_dtypes.bfloat16
    img_common = np.zeros((128, NBLOB - XCOLS), np.float32)
    whh8img = np.empty((128, WHH8_COLS), ml_dtypes.float8_e4m3)

    for d, nm in enumerate("fb"):
        wih_p = weights[f"wih_{nm}"].astype(np.float32).copy()
        whh_p = weights[f"whh_{nm}"].astype(np.float32).copy()
        bias_p = (weights[f"bih_{nm}"] + weights[f"bhh_{nm}"]).astype(np.float32).copy()
        wih_p[2 * H:3 * H] *= 2.0
        whh_p[2 * H:3 * H] *= 2.0
        bias_p[2 * H:3 * H] *= 2.0
        for g in range(GT):
            for k in range(KT):
                blkT = wih_p[g * 128:(g + 1) * 128, k * 128:(k + 1) * 128].T
                off = WIH_OFF - XCOLS + ((d * GT + g) * KT + k) * 128
                img_common[:, off:off + 128] = blkT
                blkT = whh_p[g * 128:(g + 1) * 128, k * 128:(k + 1) * 128].T
                off = ((d * GT + g) * KT + k) * 128
                whh8img[:, off:off + 128] = blkT.astype(ml_dtypes.float8_e4m3)
            img_common[:, BIAS_OFF - XCOLS + d * GT + g] = bias_p[g * 128:(g + 1) * 128]
    img_common[:, ID_OFF - XCOLS:ID_OFF - XCOLS + 128] = np.eye(128, dtype=np.float32)

    Xt = np.ascontiguousarray(np.transpose(X, (2, 0, 1)))  # (E, S, B)
    piece_cols = TOKCOLS // 4
    blobs = []
    for c in range(NCORES):
        img = np.empty((128, NBLOB), np.float32)
        xc = Xt[:, :, c * BC:(c + 1) * BC].reshape(KT, 128, TOKCOLS)
        for piece in range(4):
            for k in range(KT):
                off = piece * (XCOLS // 4) + k * piece_cols
                img[:, off:off + piece_cols] = \
                    xc[k][:, piece * piece_cols:(piece + 1) * piece_cols]
        img[:, XCOLS:] = img_common
        blobs.append(img.astype(bf))
    return blobs, whh8img


_PROGRAM_CACHE = {}


def _get_program():
    if "nc" not in _PROGRAM_CACHE:
        _PROGRAM_CACHE["nc"] = _build_program()
    return _PROGRAM_CACHE["nc"]


def _run(inputs, trace=False):
    X = np.asarray(inputs["inputs"], np.float32)
    blobs, whh8img = _pack_blob(X, inputs)
    nc = _get_program()
    in_maps = [{"blob": b, "whh8": whh8img} for b in blobs]
    res = run_bass_kernel_spmd(nc, in_maps, core_ids=list(range(NCORES)), trace=trace)
    # assemble (B, 2H): out[p, d*16 + k*8 + b] = h_d[dim 128k+p, batch b]
    emb = np.empty((B, 2 * H), np.float32)
    for c in range(NCORES):
        o = res.results[c]["out"]  # (128, 32)
        for d in range(2):
            for k in range(KT):
                blk = o[:, (d * KT + k) * BC:(d * KT + k + 1) * BC]  # (128, BC)
                emb[c * BC:(c + 1) * BC, d * H + k * 128:d * H + (k + 1) * 128] = blk.T
    return emb, res


def kernel(**inputs):
    emb, _ = _run(inputs, trace=False)
    return emb


# revision 29
# speedup vs baseline: 1.1624x; 1.0157x over previous
"""Trainium2 Bass kernel for the windowed bidirectional LSTM encoder.

Semantics (derived from the reference): each direction is a plain LSTM cell
chain over a token stream of length NT = 2S-1 (windows overlap, so tokens
repeat: fwd stream = x0,x1,x1,x2,x2,...,x511; bwd stream = x1,x0,x2,x1,...).
The output is the per-feature max of all NT hidden states of each direction,
concatenated: emb = [max_t h_f(t) | max_t h_b(t)] -> (B, 2H).

Distribution: 8 cores, each owns 8 batch rows and runs both directions.

Chunk-parallel recurrence: the LSTM map is contractive (forget gate < 0.82),
so the NT-step chain is split into K=16 chunks of L=64 steps processed in
parallel as extra "batch" columns. Each chunk is warmed up W=12 steps from a
zero state starting inside the previous chunk's range; warmed-up state error
is ~0.6^W ~ 1e-2, still below the fp8/bf16 quantization floor. Chunk 0's warmup reads a zero-padded P region, which keeps
its state exactly (0,0) until its true start.

Per-core phases (GPSIMD compute is unavailable on this image, so everything
pointwise lives on DVE + ACT):
  phase 1: P[d,g,tok,b] = bf16(Wih_d @ x_tok + bias_d) for all 512 tokens
           (weights-stationary matmuls over piece-wise DMA'd X; bias folded
           into the PSUM->SBUF copies, which alternate ACT/DVE).
  phase 2: NP = L+W parallel steps. Per step and direction:
           PE injects P[tok(chunk,step)] into PSUM via an identity matmul
           over a 32-slot-strided chunk AP (each PSUM bank sees exactly one
           start=True, its first writer, since start marks the whole bank
           pending-zero), then accumulates Whh @ h on top with fp8 DoubleRow
           matmuls (so the z-add never touches DVE); ACT runs sigmoid(i,f,g)
           on the critical path (g rows pre-scaled x2 so tanh(zg) =
           2*sig(2 zg)-1), sigmoid(o) off-path, and tanh(c); DVE does the
           bf16 gate algebra, the fp8 h for the next matmul, and the running
           max of h (warmup steps and the single tail-pad position excluded).

Recurring data is bf16 (DVE 2x mode) with fp8 h/Whh; PSUM accumulates fp32.
"""

import numpy as np
import ml_dtypes

import concourse.bass as bass
import concourse.mybir as mybir
from concourse import bacc
from concourse.tile import TileContext
from concourse.bass_utils import run_bass_kernel_spmd

F32 = mybir.dt.float32
BF16 = mybir.dt.bfloat16
FP8 = mybir.dt.float8e4
DR = mybir.MatmulPerfMode.DoubleRow
AF = mybir.ActivationFunctionType
ALU = mybir.AluOpType

S = 512
B = 64
E = 256
H = 256
NCORES = 8
BC = B // NCORES          # batch rows per core = 8
NT = 2 * S - 1            # stream steps per direction = 1023
KT = 2                    # k-tiles (contraction 256 = 2x128)
GT = 8                    # gate tiles (4H = 1024 = 8x128)

KC = 16                   # parallel chunks per direction
L = 64                    # chunk length (KC * L = 1024 >= NT)
W = 8                     # warmup steps per chunk
NP = L + W                # parallel steps
CC = KC * BC              # moving columns per direction = 128
Z = 8                     # zero-pad slots in front of P (>= ceil(W/2))
SLOTS = Z + S + 1         # P slots per (dir, gate-tile): pad + tokens + tail

TOKCOLS = S * BC          # 4096 x columns per k-tile in phase 1
CHUNK = 512               # moving cols per phase-1 matmul
NCHUNK = TOKCOLS // CHUNK # 8

# blob column layout (all bf16, 128 partitions):
XCOLS = KT * TOKCOLS                     # 8192
WIH_OFF = XCOLS                          # 2*8*2*128 = 4096  (d, g, k)
ID_OFF = WIH_OFF + 2 * GT * KT * 128     # 128
BIAS_OFF = ID_OFF + 128                  # 16
NBLOB = BIAS_OFF + 2 * GT
WHH8_COLS = 2 * GT * KT * 128            # fp8 whh, (d, g, k) blocks


def _fwd_slot(p):
    """P slot index for fwd stream position p (valid for p >= -2*Z)."""
    return Z + (p + 1) // 2


def _bwd_slot(p):
    """P slot index for bwd stream position p. Slot Z+S holds P_b[S-1] so the
    p == NT-1 cap falls out of the even-position formula."""
    if p % 2 == 0:
        return Z + p // 2 + 1
    return Z + (p - 1) // 2


def _build_program():
    import os
    debug = bool(os.environ.get("K_DEBUG"))
    nc = bacc.Bacc(None, target_bir_lowering=False)
    blob = nc.dram_tensor("blob", [128, NBLOB], BF16, kind="ExternalInput")
    whh8 = nc.dram_tensor("whh8", [128, WHH8_COLS], FP8, kind="ExternalInput")
    out = nc.dram_tensor("out", [128, 2 * KT * BC], F32, kind="ExternalOutput")
    if debug:
        dbg_p = nc.dram_tensor("dbg_p", [128, 2 * GT * SLOTS * BC], BF16, kind="ExternalOutput")
        dbg_s = nc.dram_tensor("dbg_s", [128, 4 * GT * CC], BF16, kind="ExternalOutput")
        dbg_h = nc.dram_tensor("dbg_h", [128, 16 * KT * CC], BF16, kind="ExternalOutput")
        dbg_z = nc.dram_tensor("dbg_z", [128, 4 * GT * CC], F32, kind="ExternalOutput")

    with TileContext(nc) as tc:
        with (
            tc.tile_pool(name="const", bufs=1) as const_pool,
            tc.tile_pool(name="pbuf", bufs=1) as p_pool,
            tc.tile_pool(name="sall", bufs=2) as sall_pool,
            tc.tile_pool(name="chain", bufs=2) as chain_pool,
            tc.tile_pool(name="state", bufs=2) as state_pool,
            tc.tile_pool(name="acc", bufs=1) as acc_pool,
            tc.tile_pool(name="zpsum", bufs=2, space="PSUM") as zpsum,
        ):
            blob_sb = const_pool.tile([128, NBLOB], BF16)
            # X is packed piece-major (4 pieces of 2 chunks, each piece holding
            # both k-tiles contiguously) so each DMA piece is one flat range;
            # weights/consts go first so phase-1 matmuls start early.
            nc.sync.dma_start(blob_sb[:, XCOLS:], blob[:, XCOLS:])
            PIECE = XCOLS // 4
            for piece in range(4):
                cols = slice(piece * PIECE, (piece + 1) * PIECE)
                nc.sync.dma_start(blob_sb[:, cols], blob[:, cols])

            def x_ap(k, chk):
                off = (chk // 2) * PIECE + k * (PIECE // 2) + (chk % 2) * CHUNK
                return blob_sb[:, off:off + CHUNK]

            def wih_ap(d, g, k):
                off = WIH_OFF + ((d * GT + g) * KT + k) * 128
                return blob_sb[:, off:off + 128]

            whh_sb = const_pool.tile([128, WHH8_COLS], FP8)
            nc.sync.dma_start(whh_sb[:], whh8[:])
            whh_view = whh_sb[:].rearrange("p (d g k m) -> p d g k m", d=2, g=GT, k=KT)

            ident = blob_sb[:, ID_OFF:ID_OFF + 128]

            # biases must be fp32 scalar-APs; upconvert once and pre-touch on
            # each engine that will use them as a tensor_scalar operand so the
            # dependency is already in that engine's vector clock (walrus
            # allows one sync-wait per compute instruction).
            bias_f32 = const_pool.tile([128, 2 * GT], F32)
            nc.vector.tensor_copy(bias_f32[:], blob_sb[:, BIAS_OFF:BIAS_OFF + 2 * GT])
            probe_v = const_pool.tile([128, 1], F32)
            nc.vector.tensor_copy(probe_v[:], bias_f32[:, 0:1])
            probe_a = const_pool.tile([128, 1], F32)
            nc.scalar.copy(probe_a[:], bias_f32[:, 0:1])

            def bias_ap(d, g):
                off = d * GT + g
                return bias_f32[:, off:off + 1]

            # P storage: (128, dir, gate-tile, slot, batch) bf16
            p_sb = p_pool.tile([128, 2 * GT * SLOTS * BC], BF16)
            p_view = p_sb[:].rearrange(
                "p (d g t b) -> p d g t b", d=2, g=GT, t=SLOTS, b=BC
            )

            # zero the warmup pad and the tail slot up front
            nc.vector.memset(p_view[:, :, :, 0:Z, :], 0)
            nc.vector.memset(p_view[:, :, :, Z + S, :], 0)

            # ---------------- phase 1: input projections ----------------
            # chunk-outer so compute on chunk c overlaps the DMA of chunk c+2
            for chk in range(NCHUNK):
                for d in range(2):
                    for g in range(GT):
                        zt = zpsum.tile([128, GT * CC], F32, tag=f"z{d}")
                        ps = zt[:, 0:CHUNK]
                        for k in range(KT):
                            nc.tensor.matmul(
                                ps,
                                wih_ap(d, g, k),
                                x_ap(k, chk),
                                start=(k == 0),
                                stop=(k == KT - 1),
                            )
                        dst = p_view[
                            :, d, g, Z + chk * (CHUNK // BC):Z + (chk + 1) * (CHUNK // BC), :
                        ]
                        if g % 2 == 0:
                            nc.scalar.activation(dst, ps, AF.Identity, bias=bias_ap(d, g))
                        else:
                            nc.vector.tensor_scalar(dst, ps, bias_ap(d, g), None, ALU.add)

            # bwd tail slot: stream position NT-1 maps to slot Z+S = P_b[S-1]
            nc.vector.tensor_copy(
                p_view[:, 1, :, Z + S, :], p_view[:, 1, :, Z + S - 1, :]
            )

            # ---------------- phase 2: chunk-parallel recurrence ----------------
            hmax = [acc_pool.tile([128, KT * CC], BF16, name=f"hmax{d}") for d in range(2)]
            h_prev = [None, None]
            c_prev = [None, None]
            slot_of = [_fwd_slot, _bwd_slot]

            for t in range(NP):
                deferred = []
                for d in range(2):
                    zt = zpsum.tile([128, GT * CC], F32, tag=f"z{d}", name=f"z{d}_{t}")
                    z5 = zt[:].rearrange(
                        "p (h g j b) -> p h g j b", h=2, g=GT // 2, j=KC, b=BC
                    )
                    # inject P. start=True marks the whole 2KB PSUM bank
                    # "pending zero", so each bank gets exactly ONE start=True
                    # (its first writer); later matmuls use start=False and
                    # the pending mechanism overwrites on first touch.
                    # The chunk stride is 32 slots, and slot(p0+L)-32 equals
                    # chunk 0's correct slot at every step except one warmup
                    # step per direction (the +1 rounding in the stream->token
                    # map sends the still-invalid position to token 0); that
                    # step splits chunk 0 into its own pad-slot matmul.
                    p0 = t - W
                    sl_j1 = slot_of[d](L + p0)          # chunk 1 slot
                    base = sl_j1 - 32                   # merged chunk-0 slot
                    merged = (base == slot_of[d](p0)) if p0 >= 0 else (base < Z)
                    for gh in range(2):
                        if merged:
                            rhs = p_view[
                                :, d, gh * 4:(gh + 1) * 4,
                                base:base + 32 * (KC - 1) + 1:32, :,
                            ]
                            nc.tensor.matmul(
                                z5[:, gh, :, :, :], ident, rhs,
                                start=True, stop=(t == 0),
                                skip_group_check=True,
                            )
                        else:
                            rhs_b = p_view[
                                :, d, gh * 4:(gh + 1) * 4,
                                sl_j1:sl_j1 + 32 * (KC - 2) + 1:32, :,
                            ]
                            nc.tensor.matmul(
                                z5[:, gh, :, 1:KC, :], ident, rhs_b,
                                start=True, stop=False, skip_group_check=True,
                            )
                            rhs_0 = p_view[:, d, gh * 4:(gh + 1) * 4, 0:1, :]
                            nc.tensor.matmul(
                                z5[:, gh, :, 0:1, :], ident, rhs_0,
                                start=False, stop=(t == 0), skip_group_check=True,
                            )
                    if t > 0:
                        h8 = h_prev[d]  # (128, KT, CC) fp8, k-subtile-major
                        # i/f/g tiles first so the sigmoid they feed can start
                        # before the o tiles (read by the off-path sigmoid) land
                        for g in range(GT):
                            nc.tensor.matmul(
                                zt[:, g * CC:(g + 1) * CC],
                                whh_view[:, d, g, :, :],
                                h8[:],
                                start=False,
                                stop=(g in (GT // 2 - 1, GT - 1)),
                                perf_mode=DR,
                                skip_group_check=True,
                            )

                    if debug and t < 2:
                        zc = acc_pool.tile([128, GT * CC], F32, name=f"zc{d}_{t}")
                        nc.vector.tensor_copy(zc[:], zt[:])
                        nc.sync.dma_start(
                            dbg_z[:, (t * 2 + d) * GT * CC:(t * 2 + d + 1) * GT * CC],
                            zc[:],
                        )
                    # gates: sigmoid over i,f,g (g rows pre-scaled x2) on the
                    # critical path; sigmoid over o separately (only h, late in
                    # the chain, needs it — keeps 25% of sigmoid off the path)
                    sall = sall_pool.tile([128, GT * CC], BF16, tag=f"sall{d}", name=f"sall{d}_{t}")
                    nc.scalar.activation(sall[:, 0:6 * CC], zt[:, 0:6 * CC], AF.Sigmoid)
                    nc.scalar.activation(sall[:, 6 * CC:], zt[:, 6 * CC:], AF.Sigmoid)
                    s_i = sall[:, 0:2 * CC]
                    s_f = sall[:, 2 * CC:4 * CC]
                    s_g = sall[:, 4 * CC:6 * CC]
                    s_o = sall[:, 6 * CC:8 * CC]

                    tg = chain_pool.tile([128, 2 * CC], BF16, tag=f"tg{d}", name=f"tg{d}_{t}")
                    nc.vector.tensor_scalar(tg[:], s_g, 2.0, -1.0, ALU.mult, ALU.add)
                    c_new = state_pool.tile([128, 2 * CC], BF16, tag=f"c{d}", name=f"c{d}_{t}")
                    if t == 0:
                        nc.vector.tensor_mul(c_new[:], s_i, tg[:])
                    else:
                        t1 = chain_pool.tile([128, 2 * CC], BF16, tag=f"t1{d}", name=f"t1{d}_{t}")
                        nc.vector.tensor_mul(t1[:], s_i, tg[:])
                        t2 = chain_pool.tile([128, 2 * CC], BF16, tag=f"t2{d}", name=f"t2{d}_{t}")
                        nc.vector.tensor_mul(t2[:], s_f, c_prev[d][:])
                        nc.vector.tensor_add(c_new[:], t1[:], t2[:])
                    th = chain_pool.tile([128, 2 * CC], BF16, tag=f"th{d}", name=f"th{d}_{t}")
                    nc.scalar.activation(th[:], c_new[:], AF.Tanh)
                    h_new = state_pool.tile([128, KT, CC], FP8, tag=f"h{d}", name=f"h{d}_{t}")
                    nc.vector.tensor_mul(
                        h_new[:].rearrange("p k c -> p (k c)"), s_o, th[:])
                    # defer the (off-path) h-for-max mul and running max to
                    # after both directions' on-path ops so they don't sit
                    # ahead of the other direction's chain in the in-order
                    # DVE queue; tiles stay per-direction (no coupling).
                    def _deferred_max(d=d, t=t, s_o=s_o, th=th):
                        hb = chain_pool.tile([128, 2 * CC], BF16, tag=f"hb{d}", name=f"hb{d}_{t}")
                        nc.vector.tensor_mul(hb[:], s_o, th[:])
                        if t == NP - 1:
                            # last step: chunk KC-1 sits on the single pad
                            # position past the stream end; exclude it.
                            hv = hb[:].rearrange("p (k j b) -> p k j b", k=KT, j=KC)
                            mv = hmax[d][:].rearrange("p (k j b) -> p k j b", k=KT, j=KC)
                            nc.vector.tensor_max(
                                mv[:, :, 0:KC - 1, :], mv[:, :, 0:KC - 1, :],
                                hv[:, :, 0:KC - 1, :],
                            )
                        elif t == W:
                            nc.vector.tensor_copy(hmax[d][:], hb[:])
                        else:
                            nc.vector.tensor_max(hmax[d][:], hmax[d][:], hb[:])
                        return hb
                    if t >= W:
                        deferred.append(_deferred_max)

                    h_prev[d] = h_new
                    c_prev[d] = c_new

                for fn in deferred:
                    fn()

            # ---------------- final: fold chunks, emit ----------------
            out_sb = acc_pool.tile([128, 2 * KT * BC], F32)
            for d in range(2):
                m = hmax[d][:].rearrange("p (k j b) -> p k j b", k=KT, j=KC)
                for half in (8, 4, 2, 1):
                    nc.vector.tensor_max(
                        m[:, :, 0:half, :], m[:, :, 0:half, :],
                        m[:, :, half:2 * half, :],
                    )
                nc.vector.tensor_copy(
                    out_sb[:, d * KT * BC:(d + 1) * KT * BC],
                    m[:, :, 0, :],
                )
            nc.sync.dma_start(out[:], out_sb[:])
            if debug:
                nc.sync.dma_start(dbg_p[:], p_sb[:])

    nc.compile()
    return nc


def _pack_blob(X, weights):
    """Build per-core (128, NBLOB) bf16 blobs.

    g-gate rows are pre-scaled x2 so the kernel can evaluate
    tanh(zg) = 2*sigmoid(2*zg) - 1 with the single all-gates sigmoid.
    """
    bf = ml_dtypes.bfloat16
    img_common = np.zeros((128, NBLOB - XCOLS), np.float32)
    whh8img = np.empty((128, WHH8_COLS), ml_dtypes.float8_e4m3)

    for d, nm in enumerate("fb"):
        wih_p = weights[f"wih_{nm}"].astype(np.float32).copy()
        whh_p = weights[f"whh_{nm}"].astype(np.float32).copy()
        bias_p = (weights[f"bih_{nm}"] + weights[f"bhh_{nm}"]).astype(np.float32).copy()
        wih_p[2 * H:3 * H] *= 2.0
        whh_p[2 * H:3 * H] *= 2.0
        bias_p[2 * H:3 * H] *= 2.0
        for g in range(GT):
            for k in range(KT):
                blkT = wih_p[g * 128:(g + 1) * 128, k * 128:(k + 1) * 128].T
                off = WIH_OFF - XCOLS + ((d * GT + g) * KT + k) * 128
                img_common[:, off:off + 128] = blkT
                blkT = whh_p[g * 128:(g + 1) * 128, k * 128:(k + 1) * 128].T
                off = ((d * GT + g) * KT + k) * 128
                whh8img[:, off:off + 128] = blkT.astype(ml_dtypes.float8_e4m3)
            img_common[:, BIAS_OFF - XCOLS + d * GT + g] = bias_p[g * 128:(g + 1) * 128]
    img_common[:, ID_OFF - XCOLS:ID_OFF - XCOLS + 128] = np.eye(128, dtype=np.float32)

    Xt = np.ascontiguousarray(np.transpose(X, (2, 0, 1)))  # (E, S, B)
    piece_cols = TOKCOLS // 4
    blobs = []
    for c in range(NCORES):
        img = np.empty((128, NBLOB), np.float32)
        xc = Xt[:, :, c * BC:(c + 1) * BC].reshape(KT, 128, TOKCOLS)
        for piece in range(4):
            for k in range(KT):
                off = piece * (XCOLS // 4) + k * piece_cols
                img[:, off:off + piece_cols] = \
                    xc[k][:, piece * piece_cols:(piece + 1) * piece_cols]
        img[:, XCOLS:] = img_common
        blobs.append(img.astype(bf))
    return blobs, whh8img


_PROGRAM_CACHE = {}


def _get_program():
    if "nc" not in _PROGRAM_CACHE:
        _PROGRAM_CACHE["nc"] = _build_program()
    return _PROGRAM_CACHE["nc"]


def _run(inputs, trace=False):
    X = np.asarray(inputs["inputs"], np.float32)
    blobs, whh8img = _pack_blob(X, inputs)
    nc = _get_program()
    in_maps = [{"blob": b, "whh8": whh8img} for b in blobs]
    res = run_bass_kernel_spmd(nc, in_maps, core_ids=list(range(NCORES)), trace=trace)
    # assemble (B, 2H): out[p, d*16 + k*8 + b] = h_d[dim 128k+p, batch b]
    emb = np.empty((B, 2 * H), np.float32)
    for c in range(NCORES):
        o = res.results[c]["out"]  # (128, 32)
        for d in range(2):
            for k in range(KT):
                blk = o[:, (d * KT + k) * BC:(d * KT + k + 1) * BC]  # (128, BC)
                emb[c * BC:(c + 1) * BC, d * H + k * 128:d * H + (k + 1) * 128] = blk.T
    return emb, res


def kernel(**inputs):
    emb, _ = _run(inputs, trace=False)
    return emb
